# revision 7
# baseline (speedup 1.0000x reference)
"""Trainium2 Bass kernel for nn_MyMonotoneNN_strict_dim2.

Data-parallel over 8 NeuronCores: each core handles B/8 samples.

Math (per sample b, dim d in {0,1}):
  zc      = max(z[b,d], -5);  dsz = (zc + 5) / 127
  x[n]    = (n + u[b,d,n]) * dsz - 5          (u[...,127] := 0)
  h1      = lrelu(w1*x + b1, .01)             (64 wide)
  h2      = lrelu(W2 @ h1 + b2, .01)          (64 wide)
  o[n]    = w3 . h2 + b3
  S_d     = sum_n relu(o[n]) + min(exp(o[n]), 1)      # = sum elu(o)+1
  out[b]  = dsz0*S_0 + dsz1*S_1 + bias

Device mapping:
  - column c = (sample, grid-point); both dims packed along partitions
    (dim0 -> partitions 0..63, dim1 -> 64..127).
  - L1: K=2 matmul, block-diag [w0_1 | w1_1]; the -5 shift is folded into
    the act1 bias (b1 - 5*w1); act1 = ScalarE Lrelu (alpha=.01).
  - L2: K=128 block-diag matmul; act2 folds w3 via positive-homogeneity:
    w3*lrelu(s) = lrelu_{a'}(c*s), (c, a') per output unit.
  - L3: ones-contraction via 64 partition-shifted matmuls accumulating in
    one PSUM bank -> per-point outputs land FAT [128, 512]
    (partition 2j+d = chunk j, dim d), cheap to evacuate.
  - tail: Relu/Exp passes, per-sample reduce, dsz weighting, pair-add via
    stream_shuffle, + bias.
"""

import sys

sys.path.insert(0, "/opt/trn_rl_repo")

import numpy as np

import concourse.bacc as bacc
import concourse.mybir as mybir
import concourse.tile as tile
from concourse.bass_utils import run_bass_kernel_spmd

F32 = mybir.dt.float32
AF = mybir.ActivationFunctionType
ALU = mybir.AluOpType

N_CORES = 8
NPTS = 128


def build_program(Bc, repeat=1, debug=False):
    """Per-core program for Bc samples. C = Bc*128 columns total,
    16 superchunks x 32768 cols (64 chunks of 512)."""
    assert Bc % 256 == 0
    g = Bc // 128            # samples per fat partition
    C = Bc * NPTS
    NS = Bc // 256           # superchunks
    W = 128 * g              # fat free width (per dim)
    QH = 8192                # xrow staging width (quarter superchunk)
    NQ = C // QH

    nc = bacc.Bacc("TRN2", target_bir_lowering=False, debug=False,
                   enable_asserts=False)

    z_d = nc.dram_tensor("z", [Bc, 2], F32, kind="ExternalInput").ap()
    u_d = nc.dram_tensor("u", [Bc, 2, 127], F32, kind="ExternalInput").ap()
    w1bd_d = nc.dram_tensor("w1bd", [2, 128], F32, kind="ExternalInput").ap()
    w2bd_d = nc.dram_tensor("w2bd", [128, 128], F32, kind="ExternalInput").ap()
    wbig_d = nc.dram_tensor("wbig", [128, 254], F32, kind="ExternalInput").ap()
    b1p_d = nc.dram_tensor("b1p", [128, 1], F32, kind="ExternalInput").ap()
    c2_d = nc.dram_tensor("c2", [128, 1], F32, kind="ExternalInput").ap()
    cb2_d = nc.dram_tensor("cb2", [128, 1], F32, kind="ExternalInput").ap()
    al2_d = nc.dram_tensor("al2", [128, 1], F32, kind="ExternalInput").ap()
    b3c_d = nc.dram_tensor("b3c", [128, 1], F32, kind="ExternalInput").ap()
    bias_d = nc.dram_tensor("biasv", [1, 1], F32, kind="ExternalInput").ap()
    y_d = nc.dram_tensor("y", [Bc], F32, kind="ExternalOutput").ap()
    if debug:
        xr_o = nc.dram_tensor("xr_o", [2, QH], F32, kind="ExternalOutput").ap()
        h1_o = nc.dram_tensor("h1_o", [128, 1024], F32,
                              kind="ExternalOutput").ap()
        h2_o = nc.dram_tensor("h2_o", [128, 1024], F32,
                              kind="ExternalOutput").ap()
        l3_o = nc.dram_tensor("l3_o", [128, 512], F32,
                              kind="ExternalOutput").ap()
        S_o = nc.dram_tensor("S_o", [128, 4 * NS], F32,
                             kind="ExternalOutput").ap()

    with tile.TileContext(nc) as tc:
        with (
            tc.tile_pool(name="persist", bufs=1) as pers,
            tc.tile_pool(name="xr", bufs=2) as xrpool,
            tc.tile_pool(name="h", bufs=3) as hpool,
            tc.tile_pool(name="tails", bufs=2) as tpool,
            tc.tile_pool(name="psum", bufs=3, space="PSUM") as pspool,
            tc.tile_pool(name="l3psum", bufs=2, space="PSUM") as l3pool,
        ):
            # ---- constants ----
            w1bd = pers.tile([2, 128], F32)
            nc.sync.dma_start(w1bd[:], w1bd_d[:])
            w2bd = pers.tile([128, 128], F32)
            nc.sync.dma_start(w2bd[:], w2bd_d[:])
            wbig = pers.tile([128, 254], F32)
            nc.sync.dma_start(wbig[:], wbig_d[:])
            b1p = pers.tile([128, 1], F32)
            nc.sync.dma_start(b1p[:], b1p_d[:])
            c2 = pers.tile([128, 1], F32)
            nc.sync.dma_start(c2[:], c2_d[:])
            cb2 = pers.tile([128, 1], F32)
            nc.sync.dma_start(cb2[:], cb2_d[:])
            al2 = pers.tile([128, 1], F32)
            nc.sync.dma_start(al2[:], al2_d[:])
            b3c = pers.tile([128, 1], F32)
            nc.sync.dma_start(b3c[:], b3c_d[:])
            biast = pers.tile([128, 1], F32)
            nc.sync.dma_start(biast[:], bias_d.broadcast_to([128, 1]))

            S = pers.tile([128, 4 * NS], F32)
            xfat = [None, None]

            # ---- prep: x' = (u' + n) * dsz ----
            with tc.tile_pool(name="prep", bufs=1) as prep:
                iota = prep.tile([128, W], F32)
                nc.gpsimd.iota(iota[:], pattern=[[0, g], [1, 128]],
                               channel_multiplier=0,
                               allow_small_or_imprecise_dtypes=True)
                for d in range(2):
                    uh = pers.tile([128, W], F32, tag=f"xfat{d}")
                    xfat[d] = uh
                    src = u_d[:, d, :].rearrange("(p g) n -> p g n", p=128)
                    dst = uh[:].rearrange("p (g n) -> p g n", n=128)[:, :, 0:127]
                    nc.sync.dma_start(dst, src)
                    hole = uh[:].rearrange("p (g n) -> p g n",
                                           n=128)[:, :, 127:128]
                    nc.vector.memset(hole, 0.0)

                    zd = prep.tile([128, g], F32, tag="zd")
                    nc.sync.dma_start(
                        zd[:], z_d[:, d].rearrange("(p g) -> p g", p=128))
                    dsz = prep.tile([128, g], F32, tag="dsz")
                    nc.vector.tensor_scalar(dsz[:], zd[:], -5.0, 5.0,
                                            ALU.max, ALU.add)
                    nc.vector.tensor_scalar_mul(dsz[:], dsz[:], 1.0 / 127.0)

                    nc.vector.tensor_tensor(uh[:], uh[:], iota[:], ALU.add)
                    dszb = dsz[:].broadcast_to([128, g, 128])
                    nc.vector.tensor_tensor(
                        uh[:].rearrange("p (g n) -> p g n", n=128),
                        uh[:].rearrange("p (g n) -> p g n", n=128),
                        dszb, ALU.mult)

            # fat partition p holds columns [p*W, (p+1)*W); a QH-col stage
            # slab = PF consecutive fat partitions
            PF = QH // W  # fat partitions per stage slab

            for _rep in range(repeat):
                for sc in range(NS):
                    l3ps = l3pool.tile([128, 512], F32)
                    for qh in range(32768 // QH):
                        xr = xrpool.tile([2, QH], F32, tag="xr")
                        p0 = (sc * 32768 + qh * QH) // W
                        for d in range(2):
                            nc.sync.dma_start(xr[d:d + 1, :],
                                              xfat[d][p0:p0 + PF, :])
                        if debug and sc == 0 and qh == 0:
                            nc.sync.dma_start(xr_o[:], xr[:])
                        for blk in range(QH // 1024):
                            base = blk * 1024
                            j0 = (qh * QH + blk * 1024) // 512

                            psA = pspool.tile([128, 1024], F32, tag="ps")
                            nc.tensor.matmul(psA[:, 0:512], w1bd[:],
                                             xr[:, base:base + 512],
                                             start=True, stop=True)
                            nc.tensor.matmul(psA[:, 512:1024], w1bd[:],
                                             xr[:, base + 512:base + 1024],
                                             start=True, stop=True)
                            h1 = hpool.tile([128, 1024], F32, tag="h1")
                            nc.scalar.activation(h1[:], psA[:], AF.Lrelu,
                                                 bias=b1p[:], scale=1.0,
                                                 alpha=0.01)

                            psB = pspool.tile([128, 1024], F32, tag="ps")
                            nc.tensor.matmul(psB[:, 0:512], w2bd[:],
                                             h1[:, 0:512],
                                             start=True, stop=True)
                            nc.tensor.matmul(psB[:, 512:1024], w2bd[:],
                                             h1[:, 512:1024],
                                             start=True, stop=True)
                            h2 = hpool.tile([128, 1024], F32, tag="h2")
                            nc.scalar.activation(h2[:], psB[:], AF.Prelu,
                                                 bias=cb2[:], scale=c2[:],
                                                 alpha=al2[:])
                            if debug and sc == 0 and qh == 0 and blk == 0:
                                nc.sync.dma_start(h1_o[:], h1[:])
                                nc.sync.dma_start(h2_o[:], h2[:])

                            for q in range(2):
                                j = j0 + q
                                off = 126 - 2 * j
                                nc.tensor.matmul(
                                    l3ps[:], wbig[:, off:off + 128],
                                    h2[:, q * 512:(q + 1) * 512],
                                    start=(j == 0), stop=(j == 63))

                    # ---- superchunk tail ----
                    r = tpool.tile([128, 512], F32, tag="r")
                    nc.scalar.activation(r[:], l3ps[:], AF.Relu, bias=b3c[:])
                    e = tpool.tile([128, 512], F32, tag="e")
                    nc.scalar.activation(e[:], l3ps[:], AF.Exp, bias=b3c[:])
                    if debug and sc == 0:
                        cp = tpool.tile([128, 512], F32, tag="cp")
                        nc.vector.tensor_copy(cp[:], l3ps[:])
                        nc.sync.dma_start(l3_o[:], cp[:])
                    nc.vector.tensor_scalar_min(e[:], e[:], 1.0)
                    nc.vector.tensor_tensor(r[:], r[:], e[:], ALU.add)
                    nc.vector.tensor_reduce(
                        S[:, sc * 4:(sc + 1) * 4],
                        r[:].rearrange("p (s n) -> p s n", n=128),
                        axis=mybir.AxisListType.X, op=ALU.add)

                # ---- final combine ----
                if debug:
                    nc.sync.dma_start(S_o[:], S[:])
                zd2 = tpool.tile([128, 4 * NS], F32, tag="zd2")
                for d in range(2):
                    for sc in range(NS):
                        nc.sync.dma_start(
                            zd2[d:128:2, sc * 4:(sc + 1) * 4],
                            z_d[sc * 256:(sc + 1) * 256, d].rearrange(
                                "(j i) -> j i", i=4))
                nc.vector.tensor_scalar(zd2[:], zd2[:], -5.0, 5.0,
                                        ALU.max, ALU.add)
                nc.vector.tensor_scalar_mul(zd2[:], zd2[:], 1.0 / 127.0)
                T = tpool.tile([128, 4 * NS], F32, tag="T")
                nc.vector.tensor_tensor(T[:], S[:], zd2[:], ALU.mult)
                Tsw = tpool.tile([128, 4 * NS], F32, tag="Tsw")
                mask = []
                for i in range(16):
                    mask += [2 * i + 1, 2 * i]
                nc.vector.stream_shuffle(Tsw[:], T[:], mask)
                nc.vector.tensor_tensor(T[:], T[:], Tsw[:], ALU.add)
                nc.vector.tensor_scalar(T[:], T[:], biast[:], None, ALU.add)
                for sc in range(NS):
                    nc.sync.dma_start(
                        y_d[sc * 256:(sc + 1) * 256].rearrange(
                            "(j i) -> j i", i=4),
                        T[0:128:2, sc * 4:(sc + 1) * 4])

    nc.compile()
    return nc


def _host_prep(w0_1, b0_1, w0_2, b0_2, w0_3, b0_3,
               w1_1, b1_1, w1_2, b1_2, w1_3, b1_3, bias):
    w1cat = np.concatenate([w0_1[:, 0], w1_1[:, 0]]).astype(np.float32)
    b1cat = np.concatenate([b0_1, b1_1]).astype(np.float32)
    w1bd = np.zeros((2, 128), np.float32)
    w1bd[0, :64] = w0_1[:, 0]
    w1bd[1, 64:] = w1_1[:, 0]
    b1p = (b1cat - 5.0 * w1cat).reshape(128, 1)

    w2bd = np.zeros((128, 128), np.float32)
    w2bd[:64, :64] = w0_2.T
    w2bd[64:, 64:] = w1_2.T

    w3cat = np.concatenate([w0_3[0], w1_3[0]]).astype(np.float32)
    b2cat = np.concatenate([b0_2, b1_2]).astype(np.float32)
    neg = w3cat < 0
    c2 = np.where(neg, 0.01 * w3cat, w3cat).astype(np.float32)
    al2 = np.where(neg, 100.0, 0.01).astype(np.float32)
    cb2 = (c2 * b2cat).astype(np.float32)

    b3c = np.tile(np.array([b0_3[0], b1_3[0]], np.float32), 64)

    wbig = np.zeros((128, 254), np.float32)
    wbig[0:64, 126] = 1.0
    wbig[64:128, 127] = 1.0

    return {
        "w1bd": w1bd, "w2bd": w2bd, "wbig": wbig,
        "b1p": b1p.astype(np.float32),
        "c2": c2.reshape(128, 1), "cb2": cb2.reshape(128, 1),
        "al2": al2.reshape(128, 1), "b3c": b3c.reshape(128, 1),
        "biasv": np.asarray(bias, np.float32).reshape(1, 1),
    }


_prog_cache = {}


def _get_prog(Bc, repeat=1, debug=False):
    key = (Bc, repeat, debug)
    if key not in _prog_cache:
        _prog_cache[key] = build_program(Bc, repeat, debug)
    return _prog_cache[key]


def kernel(z, u, w0_1, b0_1, w0_2, b0_2, w0_3, b0_3,
           w1_1, b1_1, w1_2, b1_2, w1_3, b1_3, bias, N, lower_bound,
           _repeat=1, _debug=False, _raw=False):
    z = np.ascontiguousarray(np.asarray(z, np.float32))
    u = np.ascontiguousarray(np.asarray(u, np.float32))
    B = z.shape[0]
    Bc = B // N_CORES
    consts = _host_prep(
        np.asarray(w0_1, np.float32), np.asarray(b0_1, np.float32),
        np.asarray(w0_2, np.float32), np.asarray(b0_2, np.float32),
        np.asarray(w0_3, np.float32), np.asarray(b0_3, np.float32),
        np.asarray(w1_1, np.float32), np.asarray(b1_1, np.float32),
        np.asarray(w1_2, np.float32), np.asarray(b1_2, np.float32),
        np.asarray(w1_3, np.float32), np.asarray(b1_3, np.float32),
        np.asarray(bias, np.float32))

    nc = _get_prog(Bc, _repeat, _debug)
    in_maps = []
    for i in range(N_CORES):
        m = dict(consts)
        m["z"] = np.ascontiguousarray(z[i * Bc:(i + 1) * Bc])
        m["u"] = np.ascontiguousarray(u[i * Bc:(i + 1) * Bc])
        in_maps.append(m)
    res = run_bass_kernel_spmd(nc, in_maps, core_ids=list(range(N_CORES)))
    if _raw:
        return res
    out = np.concatenate([res.results[i]["y"] for i in range(N_CORES)])
    return out.astype(np.float32)
